# revision 1
# baseline (speedup 1.0000x reference)
"""Trainium2 Bass kernel for nn_CCMetrics (connected-component soft-Dice).

Math
----
Reference per sample: probs = softmax(y_pred, ch axis 1) with C=2 channels,
one-hot labels y in {0,1}.  Per-voxel channel sums collapse:
  psum_v = tsum_v = 1          (softmax / one-hot sum to 1 over channels)
  inter_v = probs[true_ch] = sigmoid((2y-1) * (z1 - z0))
So per segment id k (voronoi component, 0..64):
  inter_k = sum of sigmoid values over voxels with id k
  cnt_k   = voxel count with id k
  dice_k  = (2*inter_k + eps) / (2*cnt_k + eps)
  score   = mean over present k in 1..64;  output = mean over batch.

Device algorithm (per core, data-parallel over 4M voxels / 8 cores)
------------------------------------------------------------------
Build two packed streams per voxel (id g, value v = sigmoid(...)):
  z  = g + 0.5                  (exact half-integers)
  x' = g + 0.5 + v              (value stream, thresholds at k+0.5)
Cumulative families, one instruction per bin k (per-partition accumulate):
  R_k = sum relu(x' - (k+0.5))           [ACT Relu + bias + accum]
  T_k = #{x' >= k+0.5} = #{g >= k}       [DVE tensor_scalar is_ge + accum]
  F_k = sum sigmoid(30*(z-(k+0.5)))      [ACT Sigmoid + bias + accum]
        = 0.5*cnt_k + T_{k+1}   (exact to ~1e-13: args are multiples of 30)
Recovery (host, float64):  M_k = R_k - R_{k+1} = inter_k + T_{k+1};
walking k = 64..1 with T_65 = 0: exact T anchors from DVE bins, F bins give
cnt_k = 2*(F_k - T_{k+1}).  ACT pipelines accumulate passes at ~1.3 us while
DVE accumulate passes have a ~4.3 us drain period, so ACT takes the relu
family plus most count bins (sigmoid) and DVE takes preprocessing plus a
spread subset of exact-count anchor bins.
"""

import os
import sys

import numpy as np

for _p in ("/opt/trn_rl_repo",):
    if os.path.isdir(_p) and _p not in sys.path:
        sys.path.insert(0, _p)

from concourse import bacc, bass, mybir, tile  # noqa: E402
from concourse import bass_utils  # noqa: E402

NUM_COMP = 64
EPS = 1e-5
B, C, H, W, D = 2, 2, 128, 128, 128
N = H * W * D
NCORES = 8
CORES_PER_SAMPLE = NCORES // B
CHUNK = N // CORES_PER_SAMPLE
P = 128
F = CHUNK // P
KMAX = NUM_COMP

# Exact-count anchor bins computed on DVE (tensor_scalar is_ge + accum).
# Spread so that sigmoid-chain reconstruction segments stay short.
_nd = int(os.environ.get("CC_ND", "62"))
if _nd >= KMAX:
    DVE_BINS = frozenset(range(1, KMAX + 1))
else:
    # evenly spread anchors from k=KMAX downward
    _step = max(1, round(KMAX / max(_nd, 1)))
    DVE_BINS = frozenset(
        k for k in range(KMAX, 0, -_step)
    ) | {KMAX}
    DVE_BINS = frozenset(sorted(DVE_BINS, reverse=True)[:max(_nd, 1)])
TRACE = False

_prog_cache = {}


def _build_program():
    nc = bacc.Bacc(
        "TRN2",
        target_bir_lowering=False,
        debug=False,
        enable_asserts=False,
        num_devices=NCORES,
    )
    f32 = mybir.dt.float32
    u8 = mybir.dt.uint8

    z0_d = nc.dram_tensor("z0", [P, F], f32, kind="ExternalInput").ap()
    z1_d = nc.dram_tensor("z1", [P, F], f32, kind="ExternalInput").ap()
    y_d = nc.dram_tensor("yb", [P, F], u8, kind="ExternalInput").ap()
    g_d = nc.dram_tensor("vor", [P, F], u8, kind="ExternalInput").ap()
    # bias constants: col j (j=0..63): -(j+1.5) for relu; col 64: 0.0
    negk_d = nc.dram_tensor("negk", [P, KMAX + 1], f32, kind="ExternalInput").ap()
    # sigmoid bias constants: col j: -30*(j+1.5)
    sigb_d = nc.dram_tensor("sigb", [P, KMAX], f32, kind="ExternalInput").ap()
    out_d = nc.dram_tensor("out", [P, 3 * KMAX], f32, kind="ExternalOutput").ap()

    Alu = mybir.AluOpType
    Act = mybir.ActivationFunctionType

    with tile.TileContext(nc) as tc:
        with tc.tile_pool(name="main", bufs=1) as pool:
            z0 = pool.tile([P, F], f32)
            z1 = pool.tile([P, F], f32)
            yt = pool.tile([P, F], u8)
            gt = pool.tile([P, F], u8)
            negk = pool.tile([P, KMAX + 1], f32)
            sigb = pool.tile([P, KMAX], f32)
            # small index/const tensors first: the early ACT sigmoid block
            # needs only gt/sigb, so it must not wait behind the 4MB z DMAs
            nc.sync.dma_start(out=gt[:], in_=g_d[:])
            nc.sync.dma_start(out=sigb[:], in_=sigb_d[:])
            nc.sync.dma_start(out=negk[:], in_=negk_d[:])
            nc.sync.dma_start(out=yt[:], in_=y_d[:])
            nc.sync.dma_start(out=z0[:], in_=z0_d[:])
            nc.sync.dma_start(out=z1[:], in_=z1_d[:])

            # ---- preprocessing (DVE) ----
            zt = pool.tile([P, F], f32, tag="ru5")
            nc.vector.tensor_scalar(
                out=zt[:], in0=gt[:], scalar1=0.5, scalar2=None, op0=Alu.add,
            )
            s = pool.tile([P, F], f32, tag="ru1")
            nc.vector.tensor_sub(s[:], z1[:], z0[:])
            yf = pool.tile([P, F], f32, tag="ru2")
            nc.vector.tensor_scalar(
                out=yf[:], in0=yt[:], scalar1=2.0, scalar2=-1.0,
                op0=Alu.mult, op1=Alu.add,
            )
            t = pool.tile([P, F], f32, tag="ru3")
            nc.vector.tensor_mul(t[:], s[:], yf[:])

            racc = pool.tile([P, KMAX], f32)
            tacc = pool.tile([P, KMAX], f32)
            facc = pool.tile([P, KMAX], f32)
            trash_a = pool.tile([P, F], f32)
            trash_s = pool.tile([P, F], f32, tag="ru4")

            sig_bins = [k for k in range(1, KMAX + 1) if k not in DVE_BINS]

            # a few sigmoid count passes first: they only need zt, so ACT
            # starts ~3us in while the DVE preprocessing chain runs
            def emit_sig(k):
                j = k - 1
                nc.scalar.activation(
                    out=trash_s[:], in_=zt[:], func=Act.Sigmoid,
                    bias=sigb[:, j:j + 1], scale=30.0,
                    accum_out=facc[:, j:j + 1],
                )

            head = sig_bins[:3]
            for k in head:
                emit_sig(k)
            v = pool.tile([P, F], f32)
            nc.scalar.activation(
                out=v[:], in_=t[:], func=Act.Sigmoid,
                bias=negk[:, KMAX:KMAX + 1], scale=1.0,  # bias 0.0
            )
            for k in sig_bins[3:]:
                emit_sig(k)
            x = pool.tile([P, F], f32)
            nc.vector.tensor_add(x[:], v[:], zt[:])
            # exact count anchors on DVE: is_ge at fp16 4x + 2x fold tree
            # instead of the 1x accumulate path (counts <= 2048 stay exact
            # in fp16; the final global fold level is fp32)
            f16 = mybir.dt.float16
            x16 = pool.tile([P, F], f16, tag="ru2")
            nc.vector.tensor_copy(x16[:], x[:])
            dbins = sorted(DVE_BINS)
            nbins = len(dbins)
            cmp16 = pool.tile([P, F], f16, tag="ru1")
            fb1 = pool.tile([P, F // 2], f16, tag="ru3")
            fb2 = pool.tile([P, F // 4], f16)
            fb3 = pool.tile([P, F // 8], f16)
            RW = F // 16  # 256: remnant width per bin
            remn = pool.tile([P, nbins * RW], f16, tag="ru4")
            for bi, k in enumerate(dbins):
                nc.vector.tensor_scalar(
                    out=cmp16[:], in0=x16[:], scalar1=float(k) + 0.5,
                    scalar2=None, op0=Alu.is_ge,
                )
                nc.vector.tensor_add(fb1[:], cmp16[:, :F // 2], cmp16[:, F // 2:])
                nc.vector.tensor_add(fb2[:], fb1[:, :F // 4], fb1[:, F // 4:])
                nc.vector.tensor_add(fb3[:], fb2[:, :F // 8], fb2[:, F // 8:])
                nc.vector.tensor_add(
                    remn[:, bi * RW:(bi + 1) * RW],
                    fb3[:, :F // 16], fb3[:, F // 16:])
            # global fold cascade over all bins' remnants: [P, nbins, w]
            gb = remn
            w = RW
            while w > 1:
                half = w // 2
                src = gb[:].rearrange("p (g d) -> p g d", d=w)
                dt_lvl = f16 if half >= 2 else f32  # last level bound 4096
                # cascade levels reuse slots of tiles dead by this point
                _tg = {128: "ru5", 64: "ru3", 32: "ru1", 16: "ru2"}.get(half, f"gfold{w}")
                dst_t = pool.tile([P, nbins * half], dt_lvl,
                                  name=f"gfold{w}", tag=_tg)
                dst = dst_t[:].rearrange("p (g d) -> p g d", d=half)
                nc.vector.tensor_add(dst, src[:, :, :half], src[:, :, half:])
                gb = dst_t
                w = half
            # gb is [P, nbins] with T_k per partition for dbins order
            nc.vector.tensor_copy(tacc[:, 0:nbins], gb[:])
            # relu value block on ACT (one activation-table switch total)
            for k in range(1, KMAX + 1):
                j = k - 1
                nc.scalar.activation(
                    out=trash_a[:], in_=x[:], func=Act.Relu,
                    bias=negk[:, j:j + 1], scale=1.0,
                    accum_out=racc[:, j:j + 1],
                )

            nc.sync.dma_start(out=out_d[:, 0:KMAX], in_=racc[:])
            nc.sync.dma_start(out=out_d[:, KMAX:2 * KMAX], in_=tacc[:])
            nc.sync.dma_start(out=out_d[:, 2 * KMAX:3 * KMAX], in_=facc[:])

    nc.compile()
    return nc


def _get_program():
    key = ("prog", tuple(sorted(DVE_BINS)))
    if key not in _prog_cache:
        _prog_cache[key] = _build_program()
    return _prog_cache[key]


def _consts():
    negk = np.concatenate(
        [-(np.arange(1, KMAX + 1, dtype=np.float32) + 0.5), np.zeros(1, np.float32)])
    sigb = -30.0 * (np.arange(1, KMAX + 1, dtype=np.float32) + 0.5)
    return (np.broadcast_to(negk, (P, KMAX + 1)).copy(),
            np.broadcast_to(sigb, (P, KMAX)).copy())


def kernel(y_pred: np.ndarray, y: np.ndarray, voronoi: np.ndarray) -> np.ndarray:
    y_pred = np.asarray(y_pred, dtype=np.float32)
    y = np.asarray(y)
    voronoi = np.asarray(voronoi)

    nc = _get_program()
    negk, sigb = _consts()

    in_maps = []
    for c in range(NCORES):
        b = c // CORES_PER_SAMPLE
        q = c % CORES_PER_SAMPLE
        sl = slice(q * CHUNK, (q + 1) * CHUNK)
        zp = y_pred[b].reshape(C, N)
        in_maps.append({
            "z0": np.ascontiguousarray(zp[0, sl]).reshape(P, F),
            "z1": np.ascontiguousarray(zp[1, sl]).reshape(P, F),
            "yb": np.ascontiguousarray(
                y[b, 0].reshape(N)[sl]).astype(np.uint8).reshape(P, F),
            "vor": np.ascontiguousarray(
                voronoi[b].reshape(N)[sl]).astype(np.uint8).reshape(P, F),
            "negk": negk,
            "sigb": sigb,
        })

    res = bass_utils.run_bass_kernel_spmd(
        nc, in_maps, core_ids=list(range(NCORES)), trace=TRACE,
    )
    kernel.last_results = res

    # ---- host-side gather/unshard: combine per-core partials ----
    R = np.zeros((B, KMAX + 2), dtype=np.float64)
    Tm = np.zeros((B, KMAX + 2), dtype=np.float64)
    Fm = np.zeros((B, KMAX + 2), dtype=np.float64)
    for c in range(NCORES):
        b = c // CORES_PER_SAMPLE
        out = np.asarray(res.results[c]["out"], dtype=np.float64)
        R[b, 1:KMAX + 1] += out[:, 0:KMAX].sum(axis=0)
        for bi, kk in enumerate(sorted(DVE_BINS)):
            Tm[b, kk] += out[:, KMAX + bi].sum(axis=0)
        Fm[b, 1:KMAX + 1] += out[:, 2 * KMAX:3 * KMAX].sum(axis=0)

    scores = []
    for b in range(B):
        cnt = np.zeros(KMAX + 2)
        T = np.zeros(KMAX + 2)          # reconstructed T_k, T_65 = 0
        for k in range(KMAX, 0, -1):
            if k in DVE_BINS:
                T[k] = Tm[b, k]
                cnt[k] = T[k] - T[k + 1]
            else:
                cnt[k] = 2.0 * (Fm[b, k] - T[k + 1])
                T[k] = T[k + 1] + cnt[k]
        k = np.arange(1, KMAX + 1)
        M = R[b, k] - R[b, k + 1]
        inter = M - T[k + 1]
        cntk = cnt[k]
        # counts are integers; snap to kill sigmoid-chain noise
        cntk = np.round(cntk)
        dice = (2.0 * inter + EPS) / (2.0 * cntk + EPS)
        present = cntk > 0
        n_present = max(present.sum(), 1)
        scores.append(np.where(present, dice, 0.0).sum() / n_present)

    return np.float32(np.mean(scores))



# revision 4
# speedup vs baseline: 1.5617x; 1.5617x over previous
"""Trainium2 Bass kernel for nn_CCMetrics (connected-component soft-Dice).

Math
----
Reference per sample: probs = softmax(y_pred, ch axis 1) with C=2 channels,
one-hot labels y in {0,1}.  Per-voxel channel sums collapse:
  psum_v = tsum_v = 1          (softmax / one-hot sum to 1 over channels)
  inter_v = probs[true_ch] = sigmoid((2y-1) * (z1 - z0))
So per segment id k (voronoi component, 0..64):
  inter_k = sum of sigmoid values over voxels with id k
  cnt_k   = voxel count with id k
  dice_k  = (2*inter_k + eps) / (2*cnt_k + eps)
  score   = mean over present k in 1..64;  output = mean over batch.

Device algorithm (per core, data-parallel over 4M voxels / 8 cores)
------------------------------------------------------------------
Streams (f16): zt = g + 0.5 (ids), x' = zt + v with v = sigmoid((2y-1)s).
Cumulative families over bins k = 1..64:
  T_k = #{g >= k}            R_k = sum relu(x' - (k+0.5))
Recovery (host): M_k = R_k - R_{k+1} = inter_k + T_{k+1};
cnt_k = T_k - T_{k+1}.  T_65 = R_65 = 0.

Engine split per bin (measured rates):
  PE bins:  DVE tensor_scalar stream (1.22us) + 8 accumulating ones-matmuls
            folding [128,4096]->psum row (1.73us, ldweights hidden).
            Stationary for psum row r is a shifted window of a [128,128]
            const with ones in column 64, so the whole family accumulates
            into one [64,512] psum tile, drained once.
  ACT bins: one activation+accum pass each (4.0us): Sign on zt for counts
            (sum sign = 2T - N), Relu on x' for values.
  DVE solo: is_ge mask + f16 fold tree (3.8us) for a few count bins.
"""

import os
import sys

import numpy as np

for _p in ("/opt/trn_rl_repo",):
    if os.path.isdir(_p) and _p not in sys.path:
        sys.path.insert(0, _p)

from concourse import bacc, bass, mybir, tile  # noqa: E402
from concourse import bass_utils  # noqa: E402

NUM_COMP = 64
EPS = 1e-5
B, C, H, W, D = 2, 2, 128, 128, 128
N = H * W * D
NCORES = 8
CORES_PER_SAMPLE = NCORES // B
CHUNK = N // CORES_PER_SAMPLE
P = 128
F = CHUNK // P
KMAX = NUM_COMP
NCHUNK = 8          # psum fold chunks per bin
CW = F // NCHUNK    # 512 columns per matmul

# ---- bin assignment (k = 1..64 per family) ----
ACT_CNT = frozenset(range(1, 9))            # Sign bins on ACT
ACT_VAL = frozenset(range(10, 64, 2))       # Relu bins on ACT (27)
SOLO_CNT = frozenset(range(59, 65))         # DVE mask+fold count bins (6)
PE_CNT = frozenset(range(1, KMAX + 1)) - ACT_CNT - SOLO_CNT
PE_VAL = frozenset(range(1, KMAX + 1)) - ACT_VAL

TRACE = False

_prog_cache = {}


def _build_program():
    nc = bacc.Bacc(
        "TRN2",
        target_bir_lowering=False,
        debug=False,
        enable_asserts=False,
        num_devices=NCORES,
    )
    f16 = mybir.dt.float16
    f32 = mybir.dt.float32

    zt_d = nc.dram_tensor("zt", [P, F], f16, kind="ExternalInput").ap()
    z0_d = nc.dram_tensor("z0", [P, F], f16, kind="ExternalInput").ap()
    z1_d = nc.dram_tensor("z1", [P, F], f16, kind="ExternalInput").ap()
    yf_d = nc.dram_tensor("yf", [P, F], f16, kind="ExternalInput").ap()
    # ones-column const: onec[p, c] = 1 iff c == 64
    onec_d = nc.dram_tensor("onec", [P, 2 * P], f16, kind="ExternalInput").ap()
    # ACT bias constants: col k-1: -(k+0.5) relu / -(k+0.25) sign
    rbias_d = nc.dram_tensor("rbias", [P, KMAX], f32, kind="ExternalInput").ap()
    sbias_d = nc.dram_tensor("sbias", [P, KMAX], f32, kind="ExternalInput").ap()
    # outputs
    pcnt_d = nc.dram_tensor("pcnt", [KMAX, CW], f32, kind="ExternalOutput").ap()
    pval_d = nc.dram_tensor("pval", [KMAX, CW], f32, kind="ExternalOutput").ap()
    aacc_d = nc.dram_tensor("aacc", [P, 2 * KMAX], f32, kind="ExternalOutput").ap()
    solo_d = nc.dram_tensor("solo", [P, len(SOLO_CNT)], f32,
                            kind="ExternalOutput").ap()

    Alu = mybir.AluOpType
    Act = mybir.ActivationFunctionType

    solo_bins = sorted(SOLO_CNT)

    with tile.TileContext(nc) as tc:
        with tc.tile_pool(name="main", bufs=1) as pool, \
             tc.psum_pool(name="ps", bufs=1) as ppool:
            zt = pool.tile([P, F], f16)
            z0 = pool.tile([P, F], f16)
            z1 = pool.tile([P, F], f16)
            yf = pool.tile([P, F], f16)
            onec = pool.tile([P, 2 * P], f16)
            rbias = pool.tile([P, KMAX], f32)
            sbias = pool.tile([P, KMAX], f32)
            nc.sync.dma_start(out=zt[:], in_=zt_d[:])
            nc.sync.dma_start(out=onec[:], in_=onec_d[:])
            nc.sync.dma_start(out=sbias[:], in_=sbias_d[:])
            nc.sync.dma_start(out=rbias[:], in_=rbias_d[:])
            nc.sync.dma_start(out=z0[:], in_=z0_d[:])
            nc.sync.dma_start(out=z1[:], in_=z1_d[:])
            nc.sync.dma_start(out=yf[:], in_=yf_d[:])

            ps_cnt = ppool.tile([KMAX, CW], f32)
            ps_val = ppool.tile([KMAX, CW], f32)
            aacc = pool.tile([P, 2 * KMAX], f32)   # [*,0:64] sign, [*,64:128] relu
            atr = [pool.tile([P, F], f16, name=f"atr{i}") for i in range(2)]
            masks = [pool.tile([P, F], f16, name=f"m{i}") for i in range(4)]

            # matmul plan bookkeeping: (psum tile, first/last flags)
            pe_cnt_bins = sorted(PE_CNT)
            pe_val_bins = sorted(PE_VAL)
            mm_state = {id(ps_cnt): [True, len(pe_cnt_bins) * NCHUNK],
                        id(ps_val): [True, len(pe_val_bins) * NCHUNK]}

            def fold(ps, k, m):
                st = mm_state[id(ps)]
                r = k - 1
                sta = onec[:, KMAX - r:2 * KMAX - r]
                for c in range(NCHUNK):
                    first = st[0]
                    st[0] = False
                    st[1] -= 1
                    nc.tensor.matmul(
                        out=ps[:], lhsT=sta, rhs=m[:, c * CW:(c + 1) * CW],
                        start=first, stop=(st[1] == 0),
                        skip_group_check=True,
                    )

            mi = [0]

            def next_mask():
                m = masks[mi[0] % 4]
                mi[0] += 1
                return m

            def emit_cnt_pe(k):
                m = next_mask()
                nc.vector.tensor_scalar(
                    out=m[:], in0=zt[:], scalar1=float(k) - 0.25,
                    scalar2=None, op0=Alu.is_ge,
                )
                fold(ps_cnt, k, m)

            def emit_val_pe(k, x):
                m = next_mask()
                nc.vector.tensor_scalar(
                    out=m[:], in0=x[:], scalar1=float(k) + 0.5,
                    scalar2=0.0, op0=Alu.subtract, op1=Alu.max,
                )
                fold(ps_val, k, m)

            # ---- ACT: sign count bins first (need only zt) ----
            for i, k in enumerate(sorted(ACT_CNT)):
                nc.scalar.activation(
                    out=atr[i % 2][:], in_=zt[:], func=Act.Sign,
                    bias=sbias[:, k - 1:k], scale=1.0,
                    accum_out=aacc[:, k - 1:k],
                )

            # ---- DVE: early count bins while z0/z1/yf stream in ----
            early = pe_cnt_bins[:10]
            late_cnt = pe_cnt_bins[10:]
            for k in early:
                emit_cnt_pe(k)

            # ---- preprocessing ----
            s = pool.tile([P, F], f16)
            nc.vector.tensor_sub(s[:], z1[:], z0[:])
            t = pool.tile([P, F], f16, tag="s_slot")
            nc.vector.tensor_mul(t[:], s[:], yf[:])
            v = pool.tile([P, F], f16)
            nc.scalar.activation(out=v[:], in_=t[:], func=Act.Sigmoid,
                                 bias=0.0, scale=1.0)
            x = pool.tile([P, F], f16, tag="t_slot")
            nc.vector.tensor_add(x[:], zt[:], v[:])

            # ---- ACT: relu value bins on x ----
            for i, k in enumerate(sorted(ACT_VAL)):
                nc.scalar.activation(
                    out=atr[i % 2][:], in_=x[:], func=Act.Relu,
                    bias=rbias[:, k - 1:k], scale=1.0,
                    accum_out=aacc[:, KMAX + k - 1:KMAX + k],
                )

            # ---- DVE+PE: remaining bins, interleaved ----
            nv, ncn = len(pe_val_bins), len(late_cnt)
            vi = ci = 0
            for j in range(nv + ncn):
                # interleave proportionally
                if vi * ncn <= ci * nv and vi < nv:
                    emit_val_pe(pe_val_bins[vi], x)
                    vi += 1
                elif ci < ncn:
                    emit_cnt_pe(late_cnt[ci])
                    ci += 1
                elif vi < nv:
                    emit_val_pe(pe_val_bins[vi], x)
                    vi += 1

            # ---- DVE solo count bins: is_ge + f16 fold tree ----
            cmp16 = pool.tile([P, F], f16)
            fb1 = pool.tile([P, F // 2], f16)
            fb2 = pool.tile([P, F // 4], f16)
            fb3 = pool.tile([P, F // 8], f16)
            RW = F // 16
            nso = len(solo_bins)
            remn = pool.tile([P, nso * RW], f16)
            for bi, k in enumerate(solo_bins):
                nc.vector.tensor_scalar(
                    out=cmp16[:], in0=zt[:], scalar1=float(k) - 0.25,
                    scalar2=None, op0=Alu.is_ge,
                )
                nc.vector.tensor_add(fb1[:], cmp16[:, :F // 2], cmp16[:, F // 2:])
                nc.vector.tensor_add(fb2[:], fb1[:, :F // 4], fb1[:, F // 4:])
                nc.vector.tensor_add(fb3[:], fb2[:, :F // 8], fb2[:, F // 8:])
                nc.vector.tensor_add(
                    remn[:, bi * RW:(bi + 1) * RW],
                    fb3[:, :F // 16], fb3[:, F // 16:])
            gb = remn
            w = RW
            while w > 1:
                half = w // 2
                src = gb[:].rearrange("p (g d) -> p g d", d=w)
                dt_lvl = f16 if half >= 2 else f32
                dst_t = pool.tile([P, nso * half], dt_lvl, name=f"gfold{w}")
                dst = dst_t[:].rearrange("p (g d) -> p g d", d=half)
                nc.vector.tensor_add(dst, src[:, :, :half], src[:, :, half:])
                gb = dst_t
                w = half
            solo_out = pool.tile([P, nso], f32, name="soloout")
            nc.vector.tensor_copy(solo_out[:], gb[:])

            # ---- drain ----
            stg_c = pool.tile([KMAX, CW], f32)
            stg_v = pool.tile([KMAX, CW], f32)
            nc.vector.tensor_copy(stg_c[:], ps_cnt[:])
            nc.vector.tensor_copy(stg_v[:], ps_val[:])
            nc.sync.dma_start(out=pcnt_d[:], in_=stg_c[:])
            nc.sync.dma_start(out=pval_d[:], in_=stg_v[:])
            nc.sync.dma_start(out=aacc_d[:], in_=aacc[:])
            nc.sync.dma_start(out=solo_d[:], in_=solo_out[:])

    nc.compile()
    return nc


def _get_program():
    key = "prog_v3"
    if key not in _prog_cache:
        _prog_cache[key] = _build_program()
    return _prog_cache[key]


def _consts():
    onec = np.zeros((P, 2 * P), dtype=np.float16)
    onec[:, KMAX] = 1.0
    k = np.arange(1, KMAX + 1, dtype=np.float32)
    rbias = np.broadcast_to(-(k + 0.5), (P, KMAX)).copy()
    sbias = np.broadcast_to(-(k - 0.25), (P, KMAX)).copy()
    return onec, rbias, sbias


def kernel(y_pred: np.ndarray, y: np.ndarray, voronoi: np.ndarray) -> np.ndarray:
    y_pred = np.asarray(y_pred, dtype=np.float32)
    y = np.asarray(y)
    voronoi = np.asarray(voronoi)

    nc = _get_program()
    onec, rbias, sbias = _consts()

    in_maps = []
    for c in range(NCORES):
        b = c // CORES_PER_SAMPLE
        q = c % CORES_PER_SAMPLE
        sl = slice(q * CHUNK, (q + 1) * CHUNK)
        zp = y_pred[b].reshape(C, N)
        in_maps.append({
            "zt": (voronoi[b].reshape(N)[sl].astype(np.float16)
                   + np.float16(0.5)).reshape(P, F),
            "z0": zp[0, sl].astype(np.float16).reshape(P, F),
            "z1": zp[1, sl].astype(np.float16).reshape(P, F),
            "yf": (2.0 * y[b, 0].reshape(N)[sl].astype(np.float16)
                   - np.float16(1.0)).astype(np.float16).reshape(P, F),
            "onec": onec,
            "rbias": rbias,
            "sbias": sbias,
        })

    res = bass_utils.run_bass_kernel_spmd(
        nc, in_maps, core_ids=list(range(NCORES)), trace=TRACE,
    )
    kernel.last_results = res

    solo_bins = sorted(SOLO_CNT)
    # ---- host-side gather: combine per-core partials ----
    R = np.zeros((B, KMAX + 2), dtype=np.float64)
    T = np.zeros((B, KMAX + 2), dtype=np.float64)
    for c in range(NCORES):
        b = c // CORES_PER_SAMPLE
        out = res.results[c]
        pcnt = np.asarray(out["pcnt"], dtype=np.float64)
        pval = np.asarray(out["pval"], dtype=np.float64)
        aacc = np.asarray(out["aacc"], dtype=np.float64)
        solo = np.asarray(out["solo"], dtype=np.float64)
        for k in PE_CNT:
            T[b, k] += pcnt[k - 1].sum()
        for k in PE_VAL:
            R[b, k] += pval[k - 1].sum()
        for k in ACT_CNT:
            T[b, k] += (aacc[:, k - 1].sum() + P * F) / 2.0
        for k in ACT_VAL:
            R[b, k] += aacc[:, KMAX + k - 1].sum()
        for bi, k in enumerate(solo_bins):
            T[b, k] += solo[:, bi].sum()

    scores = []
    for b in range(B):
        k = np.arange(1, KMAX + 1)
        M = R[b, k] - R[b, k + 1]          # R_65 = 0
        inter = M - T[b, k + 1]            # T_65 = 0
        cnt = T[b, k] - T[b, k + 1]
        cnt = np.round(cnt)
        dice = (2.0 * inter + EPS) / (2.0 * cnt + EPS)
        present = cnt > 0
        n_present = max(present.sum(), 1)
        scores.append(np.where(present, dice, 0.0).sum() / n_present)

    return np.float32(np.mean(scores))
